# revision 19
# baseline (speedup 1.0000x reference)
"""Trainium2 Bass kernel for GQA attention (nn_Attention_15350213116218).

B=1, S=2048, D=2048, 32 q-heads / 8 kv-heads, head_dim 64, RoPE, causal, fp32.

Sharding: tensor-parallel over heads across 8 NeuronCores. Core c gets q-heads
[4c, 4c+4) and kv-head c (wq/wk/wv column-shard, wo row-shard). Each core
computes its partial output through its wo rows; the host sums the 8 partials.

Per-core device algorithm (matmuls in bf16, fp32 PSUM accumulate):
  - All DRAM operands host-relaid-out so every DMA is a contiguous
    per-partition burst (x as [p, chunk, dblk, s], weights as [p, blk, m]).
  - Q/K/V projections computed transposed (feature-major); RoPE even/odd dims
    land in separate partition blocks via host-permuted weight columns.
  - RoPE applied in fp32 from PSUM, cast bf16 on write, DMA-interleaved into
    per-pair [h_r(32); h_i(32)] x 2 tiles; score matmuls contract K=64, two
    heads packed in the PE array via tile_position row groups.
  - softmax without max-subtraction; causal handled by restricting matmul /
    exp columns and a triangular -1e30 add on diagonal blocks.
  - P@V via lhsT = [v | ones]: ones column accumulates the softmax
    denominator; normalize with reciprocal_approx_fast + partition_broadcast.
  - Attention kb-loop is software-pipelined (scores of kb+1 issue before P@V
    of kb so the PE never waits on the exp); out_proj blocks for superblock
    N-1 are interleaved into superblock N's loop as PE gap fillers.
  - out partials written bf16 in blocked layout; host reassembles + sums.
"""
import math
import os
import sys

import numpy as np

try:
    import concourse.bass as bass
except ImportError:
    sys.path.insert(0, "/opt/trn_rl_repo")
    import concourse.bass as bass

import concourse.mybir as mybir
import concourse.tile as tile
import concourse.bass_utils as bass_utils
from concourse import bacc
from concourse.masks import make_identity, make_lower_triangular

f32 = mybir.dt.float32
f32r = mybir.dt.float32r
bf16 = mybir.dt.bfloat16

S = 2048
D = 2048
NH, NKV, HD = 32, 8, 64
NCORES = 8
HPC = NH // NCORES          # 4 q heads per core
D2 = HD // 2                # 32
P = 128
SCH = 512                   # s-chunk for projections
QSB = 512                   # q superblock for attention
NSCH = S // SCH             # 4
NQSB = S // QSB             # 4
NDBLK = D // P              # 16
NSBLK = S // P              # 16
SCALE = 1.0 / math.sqrt(HD)
NWARM = 10                  # dummy matmuls to warm the PE HAM clock gate


def _build_kernel(reps=1, phases="ABEPNC"):
    nc = bacc.Bacc("TRN2", target_bir_lowering=False)

    xt_d = nc.dram_tensor("xT", [P, NSCH, NDBLK, SCH], bf16,
                          kind="ExternalInput").ap()
    wqr_d = nc.dram_tensor("wq_r", [P, NDBLK, P], bf16, kind="ExternalInput").ap()
    wqi_d = nc.dram_tensor("wq_i", [P, NDBLK, P], bf16, kind="ExternalInput").ap()
    wkvi_d = nc.dram_tensor("wkvi", [P, NDBLK, P], bf16, kind="ExternalInput").ap()
    wo_d = nc.dram_tensor("wo_c", [P, 2, D], bf16, kind="ExternalInput").ap()
    cos_d = nc.dram_tensor("cosT4", [P, S], bf16, kind="ExternalInput").ap()
    sin_d = nc.dram_tensor("sinT4", [P, S], bf16, kind="ExternalInput").ap()
    # out blocked: [sblock, dchunk, 128, 512] bf16 partials
    out_d = nc.dram_tensor("out", [NSBLK, 4, P, 512], bf16,
                           kind="ExternalOutput").ap()

    with tile.TileContext(nc) as tc:
        for r in range(reps):
            _body(tc, xt_d, wqr_d, wqi_d, wkvi_d, wo_d, cos_d, sin_d, out_d,
                  pfx=f"r{r}_" if reps > 1 else "", phases=phases)
    nc.compile()
    return nc


def _body(tc, xt_d, wqr_d, wqi_d, wkvi_d, wo_d, cos_d, sin_d, out_d, pfx="",
          phases="ABEPNC"):
    nc = tc.nc
    Exp = mybir.ActivationFunctionType.Exp

    with (
        tc.tile_pool(name=pfx + "consts", bufs=1) as consts,
        tc.tile_pool(name=pfx + "persist", bufs=1) as persist,
    ):
        _body_inner(tc, nc, Exp, consts, persist, xt_d, wqr_d, wqi_d, wkvi_d,
                    wo_d, cos_d, sin_d, out_d, pfx, phases)


def _body_inner(tc, nc, Exp, consts, persist, xt_d, wqr_d, wqi_d, wkvi_d,
                wo_d, cos_d, sin_d, out_d, pfx, phases="ABEPNC"):
    # ---- weight / table DMAs first: issue before any const compute so the
    # queues start pulling from HBM immediately; first 4 d-blocks of each
    # weight go first so the first projection matmuls can start early ----
    wq_r = consts.tile([P, NDBLK, P], bf16, tag="wq_r")
    nc.sync.dma_start(wq_r[:, 0:4, :], wqr_d[:, 0:4])
    wq_i = consts.tile([P, NDBLK, P], bf16, tag="wq_i")
    nc.scalar.dma_start(wq_i[:, 0:4, :], wqi_d[:, 0:4])
    wkvi = consts.tile([P, NDBLK, P], bf16, tag="wkvi")
    nc.gpsimd.dma_start(wkvi[:, 0:4, :], wkvi_d[:, 0:4])
    nc.sync.dma_start(wq_r[:, 4:16, :], wqr_d[:, 4:16])
    nc.scalar.dma_start(wq_i[:, 4:16, :], wqi_d[:, 4:16])
    nc.gpsimd.dma_start(wkvi[:, 4:16, :], wkvi_d[:, 4:16])
    cosT4 = consts.tile([P, S], bf16, tag="cosT4")
    nc.gpsimd.dma_start(cosT4[:], cos_d[:])
    sinT4 = consts.tile([P, S], bf16, tag="sinT4")
    nc.gpsimd.dma_start(sinT4[:], sin_d[:])
    # wo needed only in out_proj; loaded behind x chunk 0 on the scalar queue
    wo_sb = consts.tile([P, 2, D], bf16, tag="wo_sb")

    # ---- warmup constants via pure DVE memsets (no gpsimd dependency) so
    # the warm matmuls start immediately ----
    zeros32 = consts.tile([P, 1], f32, tag="zeros32")
    nc.vector.memset(zeros32[:], 0.0)
    zeros_r = consts.tile([P, SCH], bf16, tag="zeros_r")
    nc.vector.tensor_copy(zeros_r[:], zeros32[:].to_broadcast((P, SCH)))
    ones32 = consts.tile([P, 1], f32, tag="ones32")
    nc.vector.memset(ones32[:], 1.0)
    warm_w = consts.tile([P, P], bf16, tag="warm_w")
    nc.vector.memset(warm_w[:], 1.0)

    # ---- remaining constants ----
    ident = consts.tile([P, P], f32r, tag="ident")
    ident32 = consts.tile([P, P], f32, tag="ident32")
    make_identity(nc, ident32[:])
    nc.vector.tensor_copy(ident[:], ident32[:])
    maskT = consts.tile([P, P], f32, tag="maskT")   # [k,q]: 1 where k > q
    make_lower_triangular(nc, maskT[:], val=1.0, diag=False)

    # warmup stream: keep the PE HAM clock gate busy during the DMA prologue
    with tc.tile_pool(name=pfx + "warmps", bufs=1, space="PSUM") as warmps:
        ps_w = warmps.tile([P, SCH], f32, tag="ps_w")
        for _w in range(NWARM):
            nc.tensor.matmul(ps_w[:], warm_w[:], zeros_r[:],
                             start=True, stop=True)
        # pin the warm chain against dead-code elimination: writes 0.0 over
        # maskT[0,0] which is 0.0 anyway (diagonal is unmasked); tri01 below
        # reads maskT, keeping this copy (and thus the warm matmuls) alive
        nc.vector.tensor_copy(maskT[0:1, 0:1], ps_w[0:1, 0:1])

    tri01 = consts.tile([P, P], bf16, tag="tri01")  # [k,q]: 0 where k > q
    nc.vector.tensor_scalar(tri01[:], maskT[:], -1.0, 1.0,
                            op0=mybir.AluOpType.mult,
                            op1=mybir.AluOpType.add)

    # ---- persistent activations ----
    # qp{pr}: [h_{2pr} r(32); h_{2pr} i(32); h_{2pr+1} r(32); h_{2pr+1} i(32)]
    qp0 = persist.tile([P, S], bf16, tag="qp0")
    qp1 = persist.tile([P, S], bf16, tag="qp1")
    k2 = persist.tile([P, S], bf16, tag="k2")        # [k_r; k_i] x2
    v_ones = persist.tile([P, NSBLK, HD + 1], bf16, tag="v_ones")  # [k, kb, 65]
    vT_sb = persist.tile([64, S], f32r, tag="vT_sb")
    attn_T0 = persist.tile([P, S], bf16, tag="attn_T0")  # heads 0,1
    attn_T1 = persist.tile([P, S], bf16, tag="attn_T1")  # heads 2,3

    nc.vector.tensor_copy(v_ones[:, :, HD:HD + 1],
                          ones32[:, None, :].to_broadcast((P, NSBLK, 1)))

    # ================= Phase A: QKV projections + rope =================
    if "A" not in phases:
        return
    with (
        tc.tile_pool(name=pfx + "xtsb", bufs=2) as xt_pool,
        tc.tile_pool(name=pfx + "ropetmp", bufs=2) as rtmp_pool,
        tc.tile_pool(name=pfx + "qstage", bufs=2) as qst_pool,
        tc.tile_pool(name=pfx + "psA", bufs=2, space="PSUM") as psA,
        tc.tile_pool(name=pfx + "psAq", bufs=2, space="PSUM") as psAq,
    ):
        for sch in range(NSCH):
            s0 = sch * SCH
            xt_c = xt_pool.tile([P, NDBLK, SCH], bf16, tag="xt_c")
            if sch == 0:
                # quarters: finer granularity so the first matmuls start early
                nc.sync.dma_start(xt_c[:, 0:4, :], xt_d[:, 0, 0:4])
                nc.scalar.dma_start(xt_c[:, 4:8, :], xt_d[:, 0, 4:8])
                nc.sync.dma_start(xt_c[:, 8:12, :], xt_d[:, 0, 8:12])
                nc.scalar.dma_start(xt_c[:, 12:16, :], xt_d[:, 0, 12:16])
                nc.scalar.dma_start(wo_sb[:], wo_d[:])
            else:
                nc.sync.dma_start(xt_c[:, 0:8, :], xt_d[:, sch, 0:8])
                nc.scalar.dma_start(xt_c[:, 8:16, :], xt_d[:, sch, 8:16])

            ps_q = psAq.tile([P, 2, SCH], f32, tag="ps_q")
            ps_kv = psAq.tile([P, SCH], f32, tag="ps_kv")
            for db in range(NDBLK):
                st = db == 0
                sp = db == NDBLK - 1
                nc.tensor.matmul(ps_q[:, 0, :], wq_r[:, db, :], xt_c[:, db, :],
                                 start=st, stop=sp)
                nc.tensor.matmul(ps_q[:, 1, :], wq_i[:, db, :], xt_c[:, db, :],
                                 start=st, stop=sp)
                nc.tensor.matmul(ps_kv[:], wkvi[:, db, :], xt_c[:, db, :],
                                 start=st, stop=sp)

            ssl = slice(s0, s0 + SCH)
            ps_qr = ps_q[:, 0, :]
            ps_qi = ps_q[:, 1, :]
            # ---- rope q (split layout: all 4 heads' r parts / i parts) ----
            qst_r = qst_pool.tile([P, SCH], bf16, tag="qst_r")
            qst_i = qst_pool.tile([P, SCH], bf16, tag="qst_i")
            ta = rtmp_pool.tile([P, SCH], f32, tag="ta")
            tb = rtmp_pool.tile([P, SCH], f32, tag="tb")
            nc.vector.tensor_mul(ta[:], ps_qr, cosT4[:, ssl])
            nc.vector.tensor_mul(tb[:], ps_qi, sinT4[:, ssl])
            nc.vector.tensor_sub(qst_r[:], ta[:], tb[:])
            tc2 = rtmp_pool.tile([P, SCH], f32, tag="tc2")
            td = rtmp_pool.tile([P, SCH], f32, tag="td")
            nc.vector.tensor_mul(tc2[:], ps_qr, sinT4[:, ssl])
            nc.vector.tensor_mul(td[:], ps_qi, cosT4[:, ssl])
            nc.vector.tensor_add(qst_i[:], tc2[:], td[:])
            # interleave into qp tiles: [h r; h i] per head (DMA)
            for h in range(HPC):
                qp = qp0 if h < 2 else qp1
                b = 64 * (h % 2)
                nc.gpsimd.dma_start(qp[b:b + 32, ssl],
                                    qst_r[32 * h:32 * h + 32, :])
                nc.gpsimd.dma_start(qp[b + 32:b + 64, ssl],
                                    qst_i[32 * h:32 * h + 32, :])

            # ---- rope k (kv psum rows 0:64 = [k_e, k_o]) ----
            kr = rtmp_pool.tile([32, SCH], f32, tag="kr")
            ki = rtmp_pool.tile([32, SCH], f32, tag="ki")
            nc.vector.tensor_copy(kr[:], ps_kv[0:32, :])
            nc.vector.tensor_copy(ki[:], ps_kv[32:64, :])
            tka = rtmp_pool.tile([32, SCH], f32, tag="tka")
            tkb = rtmp_pool.tile([32, SCH], f32, tag="tkb")
            nc.vector.tensor_mul(tka[:], kr[:], cosT4[0:32, ssl])
            nc.vector.tensor_mul(tkb[:], ki[:], sinT4[0:32, ssl])
            nc.vector.tensor_sub(k2[0:32, ssl], tka[:], tkb[:])
            nc.vector.tensor_mul(tka[:], kr[:], sinT4[0:32, ssl])
            nc.vector.tensor_mul(tkb[:], ki[:], cosT4[0:32, ssl])
            nc.vector.tensor_add(k2[32:64, ssl], tka[:], tkb[:])
            # replicate [k_r; k_i] to rows 64:128 (DMA)
            nc.gpsimd.dma_start(k2[64:128, ssl], k2[0:64, ssl])
            # stash vT
            nc.scalar.copy(vT_sb[:, ssl], ps_kv[64:128, :])
            # ---- v natural ([k,65] blocks with ones col) via PE transpose ----
            for kb in range(4 * sch, 4 * sch + 4):
                ps_v = psA.tile([P, 64], f32, tag="ps_v")
                nc.tensor.transpose(ps_v[:].bitcast(f32r),
                                    vT_sb[:, kb * P:(kb + 1) * P],
                                    ident[0:64, 0:64])
                nc.vector.tensor_copy(v_ones[:, kb, 0:HD], ps_v[:])

    # ====== Phase B: attention, software-pipelined, out_proj interleaved ======
    if "B" not in phases:
        return
    engs = [nc.sync, nc.scalar, nc.gpsimd]
    with (
        tc.tile_pool(name=pfx + "expt", bufs=3) as exp_pool,
        tc.tile_pool(name=pfx + "norm", bufs=4) as norm_pool,
        tc.tile_pool(name=pfx + "osb", bufs=4) as out_pool,
        tc.tile_pool(name=pfx + "psB", bufs=2, space="PSUM") as psB,
        tc.tile_pool(name=pfx + "psBo", bufs=3, space="PSUM") as psBo,
        tc.tile_pool(name=pfx + "psC", bufs=1, space="PSUM") as psC,
    ):
        def emit_c_block(qsb_c, j, pool):
            sb = 4 * qsb_c + j // 4
            dmc = j % 4
            ssl = slice(sb * P, (sb + 1) * P)
            dsl = slice(dmc * 512, (dmc + 1) * 512)
            ps_o = pool.tile([P, 512], f32, tag="ps_o", name=f"ps_o{sb}_{dmc}")
            nc.tensor.matmul(ps_o[:], attn_T0[:, ssl], wo_sb[:, 0, dsl],
                             start=True, stop=False)
            nc.tensor.matmul(ps_o[:], attn_T1[:, ssl], wo_sb[:, 1, dsl],
                             start=False, stop=True)
            osb = out_pool.tile([P, 512], bf16, tag="osb", name=f"osb{sb}_{dmc}")
            if j % 2 == 0:
                nc.scalar.copy(osb[:], ps_o[:])
            else:
                nc.vector.tensor_copy(osb[:], ps_o[:])
            engs[(4 * sb + dmc) % 3].dma_start(out_d[sb, dmc], osb[:])

        for qsb in range(NQSB):
            q0 = qsb * QSB
            nkb = (q0 + QSB) // P
            qsl = slice(q0, q0 + QSB)
            n_c = 16 if qsb > 0 else 0   # out_proj blocks of superblock qsb-1
            ci = 0
            for pr in range(2):                     # head pairs (0,1), (2,3)
                qp = qp0 if pr == 0 else qp1
                outps = [psBo.tile([HD + 1, QSB], f32, tag="outp",
                                   name=f"outp{qsb}_{pr}_{_m}") for _m in range(2)]
                prev = None                         # (kb, off, expT)
                for kb in range(nkb):
                    k0 = kb * P
                    ksl = slice(k0, k0 + P)
                    off = max(0, k0 - q0)
                    diag = k0 - q0 >= 0
                    # causal: columns [0:off] are strictly above the diagonal
                    scT = psB.tile([P, 2, QSB], f32, tag="scT")
                    for m in range(2):
                        rp = slice(64 * m, 64 * m + 64)
                        nc.tensor.matmul(scT[:, m, off:], k2[rp, ksl],
                                         qp[rp, q0 + off:q0 + QSB],
                                         start=True, stop=True,
                                         tile_position=(64 * m, 0))
                    expT = exp_pool.tile([P, 2, QSB], bf16, tag="expT")
                    nc.scalar.activation(expT[:, :, off:], scT[:, :, off:],
                                         Exp, scale=SCALE)
                    if diag:
                        # zero the strictly-upper triangle of the diagonal
                        # 128-col square (gpsimd, off the exp critical path)
                        nc.gpsimd.tensor_mul(
                            expT[:, :, off:off + P], expT[:, :, off:off + P],
                            tri01[:, None, :].to_broadcast((P, 2, P)))
                    # PE gap filler while the exp for this kb runs on ACT
                    if ci < n_c:
                        emit_c_block(qsb - 1, ci, psC)
                        ci += 1
                    if prev is not None:
                        pkb, poff, pexp = prev
                        for m in range(2):
                            nc.tensor.matmul(outps[m][:, poff:],
                                             v_ones[:, pkb, :],
                                             pexp[:, m, poff:],
                                             start=(pkb == 0),
                                             stop=False,
                                             skip_group_check=True)
                    prev = (kb, off, expT)
                pkb, poff, pexp = prev
                for m in range(2):
                    nc.tensor.matmul(outps[m][:, poff:], v_ones[:, pkb, :],
                                     pexp[:, m, poff:],
                                     start=(pkb == 0), stop=True,
                                     skip_group_check=True)
                # normalize + place into attn_T
                for m in range(2):
                    lrow = norm_pool.tile([1, QSB], f32, tag="lrow")
                    nc.vector.tensor_copy(lrow[:], outps[m][HD:HD + 1, :])
                    recip = norm_pool.tile([1, QSB], f32, tag="recip")
                    nc.vector.reciprocal_approx_fast(recip[:], lrow[:])
                    bcast = norm_pool.tile([64, QSB], f32, tag="bcast")
                    nc.gpsimd.partition_broadcast(bcast[:], recip[:])
                    dst = attn_T0 if pr == 0 else attn_T1
                    rsl = slice(64 * m, 64 * m + 64)
                    nc.vector.tensor_mul(dst[rsl, qsl], outps[m][0:HD, :],
                                         bcast[:])
            while ci < n_c:
                emit_c_block(qsb - 1, ci, psC)
                ci += 1

    # ---- out_proj tail: last superblock's blocks, own deeper psum pool ----
    with (
        tc.tile_pool(name=pfx + "osb2", bufs=4) as out_pool,
        tc.tile_pool(name=pfx + "psC2", bufs=4, space="PSUM") as psC2,
    ):
        def emit_c_tail(j):
            sb = 4 * (NQSB - 1) + j // 4
            dmc = j % 4
            ssl = slice(sb * P, (sb + 1) * P)
            dsl = slice(dmc * 512, (dmc + 1) * 512)
            ps_o = psC2.tile([P, 512], f32, tag="ps_o", name=f"ps_oT{sb}_{dmc}")
            nc.tensor.matmul(ps_o[:], attn_T0[:, ssl], wo_sb[:, 0, dsl],
                             start=True, stop=False)
            nc.tensor.matmul(ps_o[:], attn_T1[:, ssl], wo_sb[:, 1, dsl],
                             start=False, stop=True)
            osb = out_pool.tile([P, 512], bf16, tag="osb", name=f"osbT{sb}_{dmc}")
            if dmc % 2 == 0:
                nc.vector.tensor_copy(osb[:], ps_o[:])
            else:
                nc.scalar.copy(osb[:], ps_o[:])
            engs[(4 * sb + dmc) % 3].dma_start(out_d[sb, dmc], osb[:])

        for j in range(16):
            emit_c_tail(j)
    tc.strict_bb_all_engine_barrier()


_NC_CACHE = {}


def _get_nc(reps=1, phases="ABEPNC"):
    key = (reps, phases)
    if key not in _NC_CACHE:
        _NC_CACHE[key] = _build_kernel(reps, phases)
    return _NC_CACHE[key]


def _make_in_maps(x, wq, wk, wv, wo, freqs_cos, freqs_sin):
    import ml_dtypes
    bf = ml_dtypes.bfloat16
    x2 = np.asarray(x, dtype=np.float32).reshape(S, D)
    # [D, S] -> [p, sch, o, s] fully contiguous per partition
    xT = np.ascontiguousarray(
        x2.T.reshape(NDBLK, P, NSCH, SCH).transpose(1, 2, 0, 3).astype(bf))
    cos = np.asarray(freqs_cos, dtype=np.float32)
    sin = np.asarray(freqs_sin, dtype=np.float32)
    cosT4 = np.ascontiguousarray(np.tile(cos.T, (HPC, 1)).astype(bf))  # [128,S]
    sinT4 = np.ascontiguousarray(np.tile(sin.T, (HPC, 1)).astype(bf))
    wq = np.asarray(wq, dtype=np.float32)
    wk = np.asarray(wk, dtype=np.float32)
    wv = np.asarray(wv, dtype=np.float32)
    wo = np.asarray(wo, dtype=np.float32)

    def _blk(w):  # [D, 128] -> [p, o, m]
        return np.ascontiguousarray(
            w.reshape(NDBLK, P, P).transpose(1, 0, 2).astype(bf))

    in_maps = []
    for c in range(NCORES):
        wq_c = wq.reshape(D, NH, HD)[:, HPC * c:HPC * (c + 1), :]
        wq_r = _blk(wq_c[:, :, 0::2].reshape(D, HPC * D2))
        wq_i = _blk(wq_c[:, :, 1::2].reshape(D, HPC * D2))
        wk_c = wk.reshape(D, NKV, HD)[:, c, :]
        wv_c = wv.reshape(D, NKV, HD)[:, c, :]
        wkvi = _blk(np.concatenate([wk_c[:, 0::2], wk_c[:, 1::2], wv_c], axis=1))
        wo_c = np.ascontiguousarray(
            wo.reshape(NH, HD, D)[HPC * c:HPC * (c + 1)]
            .reshape(2, P, D).astype(bf).transpose(1, 0, 2))
        in_maps.append({
            "xT": xT, "wq_r": wq_r, "wq_i": wq_i, "wkvi": wkvi,
            "wo_c": wo_c, "cosT4": cosT4, "sinT4": sinT4,
        })
    return in_maps


_last_in_maps = None


def kernel(x, wq, wk, wv, wo, freqs_cos, freqs_sin, mask):
    global _last_in_maps
    in_maps = _make_in_maps(x, wq, wk, wv, wo, freqs_cos, freqs_sin)
    _last_in_maps = in_maps
    nc = _get_nc()
    res = bass_utils.run_bass_kernel_spmd(nc, in_maps, core_ids=list(range(NCORES)))
    out = np.zeros((NSBLK, 4, P, 512), dtype=np.float64)
    for r in res.results:
        out += r["out"].astype(np.float64)
    out = out.transpose(0, 2, 1, 3).reshape(S, D)
    return out.astype(np.float32).reshape(1, S, D)


# revision 22
# speedup vs baseline: 1.7213x; 1.7213x over previous
"""Trainium2 Bass kernel for GQA attention (nn_Attention_15350213116218).

B=1, S=2048, D=2048, 32 q-heads / 8 kv-heads, head_dim 64, RoPE, causal, fp32.

Sharding: tensor-parallel over heads across 8 NeuronCores. Core c gets q-heads
[4c, 4c+4) and kv-head c (wq/wk/wv column-shard, wo row-shard). Each core
computes its partial output through its wo rows; the host sums the 8 partials.

Per-core device algorithm (matmuls in bf16, fp32 PSUM accumulate):
  - All DRAM operands host-relaid-out so every DMA is a contiguous
    per-partition burst (x as [p, chunk, dblk, s], weights as [p, blk, m]).
  - Q/K/V projections computed transposed (feature-major); RoPE even/odd dims
    land in separate partition blocks via host-permuted weight columns.
  - RoPE applied in fp32 from PSUM, cast bf16 on write, DMA-interleaved into
    per-pair [h_r(32); h_i(32)] x 2 tiles; score matmuls contract K=64, two
    heads packed in the PE array via tile_position row groups.
  - softmax without max-subtraction; causal handled by restricting matmul /
    exp columns and a triangular -1e30 add on diagonal blocks.
  - P@V via lhsT = [v | ones]: ones column accumulates the softmax
    denominator; normalize with reciprocal_approx_fast + partition_broadcast.
  - Attention kb-loop is software-pipelined (scores of kb+1 issue before P@V
    of kb so the PE never waits on the exp); out_proj blocks for superblock
    N-1 are interleaved into superblock N's loop as PE gap fillers.
  - out partials written bf16 in blocked layout; host reassembles + sums.
"""
import math
import os
import sys

import numpy as np

try:
    import concourse.bass as bass
except ImportError:
    sys.path.insert(0, "/opt/trn_rl_repo")
    import concourse.bass as bass

import concourse.mybir as mybir
import concourse.tile as tile
import concourse.bass_utils as bass_utils
from concourse import bacc
from concourse.masks import make_identity, make_lower_triangular

f32 = mybir.dt.float32
f32r = mybir.dt.float32r
bf16 = mybir.dt.bfloat16

S = 2048
D = 2048
NH, NKV, HD = 32, 8, 64
NCORES = 8
HPC = NH // NCORES          # 4 q heads per core
D2 = HD // 2                # 32
P = 128
SCH = 512                   # s-chunk for projections
QSB = 512                   # q superblock for attention
NSCH = S // SCH             # 4
NQSB = S // QSB             # 4
NDBLK = D // P              # 16
NSBLK = S // P              # 16
SCALE = 1.0 / math.sqrt(HD)
NWARM = 10                  # dummy matmuls to warm the PE HAM clock gate


def _build_kernel(reps=1, phases="ABEPNC"):
    nc = bacc.Bacc("TRN2", target_bir_lowering=False)

    xt_d = nc.dram_tensor("xT", [P, NSCH, NDBLK, SCH], bf16,
                          kind="ExternalInput").ap()
    wqr_d = nc.dram_tensor("wq_r", [P, NDBLK, P], bf16, kind="ExternalInput").ap()
    wqi_d = nc.dram_tensor("wq_i", [P, NDBLK, P], bf16, kind="ExternalInput").ap()
    wkvi_d = nc.dram_tensor("wkvi", [P, NDBLK, P], bf16, kind="ExternalInput").ap()
    wo_d = nc.dram_tensor("wo_c", [P, 2, D], bf16, kind="ExternalInput").ap()
    cos_d = nc.dram_tensor("cosT4", [P, S], bf16, kind="ExternalInput").ap()
    sin_d = nc.dram_tensor("sinT4", [P, S], bf16, kind="ExternalInput").ap()
    # out blocked: [sblock, dchunk, 128, 512] bf16 partials
    out_d = nc.dram_tensor("out", [NSBLK, 4, P, 512], bf16,
                           kind="ExternalOutput").ap()

    with tile.TileContext(nc) as tc:
        for r in range(reps):
            _body(tc, xt_d, wqr_d, wqi_d, wkvi_d, wo_d, cos_d, sin_d, out_d,
                  pfx=f"r{r}_" if reps > 1 else "", phases=phases)
    nc.compile()
    return nc


def _body(tc, xt_d, wqr_d, wqi_d, wkvi_d, wo_d, cos_d, sin_d, out_d, pfx="",
          phases="ABEPNC"):
    nc = tc.nc
    Exp = mybir.ActivationFunctionType.Exp

    with (
        tc.tile_pool(name=pfx + "consts", bufs=1) as consts,
        tc.tile_pool(name=pfx + "persist", bufs=1) as persist,
    ):
        _body_inner(tc, nc, Exp, consts, persist, xt_d, wqr_d, wqi_d, wkvi_d,
                    wo_d, cos_d, sin_d, out_d, pfx, phases)


def _body_inner(tc, nc, Exp, consts, persist, xt_d, wqr_d, wqi_d, wkvi_d,
                wo_d, cos_d, sin_d, out_d, pfx, phases="ABEPNC"):
    # ---- weight / table DMAs first: issue before any const compute so the
    # queues start pulling from HBM immediately; first 4 d-blocks of each
    # weight go first so the first projection matmuls can start early ----
    wq_r = consts.tile([P, NDBLK, P], bf16, tag="wq_r")
    nc.sync.dma_start(wq_r[:, 0:4, :], wqr_d[:, 0:4])
    wq_i = consts.tile([P, NDBLK, P], bf16, tag="wq_i")
    nc.scalar.dma_start(wq_i[:, 0:4, :], wqi_d[:, 0:4])
    wkvi = consts.tile([P, NDBLK, P], bf16, tag="wkvi")
    nc.gpsimd.dma_start(wkvi[:, 0:4, :], wkvi_d[:, 0:4])
    nc.sync.dma_start(wq_r[:, 4:16, :], wqr_d[:, 4:16])
    nc.scalar.dma_start(wq_i[:, 4:16, :], wqi_d[:, 4:16])
    nc.gpsimd.dma_start(wkvi[:, 4:16, :], wkvi_d[:, 4:16])
    cosT4 = consts.tile([P, S], bf16, tag="cosT4")
    nc.gpsimd.dma_start(cosT4[:], cos_d[:])
    sinT4 = consts.tile([P, S], bf16, tag="sinT4")
    nc.gpsimd.dma_start(sinT4[:], sin_d[:])
    # wo needed only in out_proj; loaded behind x chunk 0 on the scalar queue
    wo_sb = consts.tile([P, 2, D], bf16, tag="wo_sb")

    # ---- warmup constants via pure DVE memsets (no gpsimd dependency) so
    # the warm matmuls start immediately ----
    zeros32 = consts.tile([P, 1], f32, tag="zeros32")
    nc.vector.memset(zeros32[:], 0.0)
    zeros_r = consts.tile([P, SCH], bf16, tag="zeros_r")
    nc.vector.tensor_copy(zeros_r[:], zeros32[:].to_broadcast((P, SCH)))
    ones32 = consts.tile([P, 1], f32, tag="ones32")
    nc.vector.memset(ones32[:], 1.0)
    warm_w = consts.tile([P, P], bf16, tag="warm_w")
    nc.vector.memset(warm_w[:], 1.0)

    # ---- remaining constants ----
    ident = consts.tile([P, P], f32r, tag="ident")
    ident32 = consts.tile([P, P], f32, tag="ident32")
    make_identity(nc, ident32[:])
    nc.vector.tensor_copy(ident[:], ident32[:])
    maskT = consts.tile([P, P], f32, tag="maskT")   # [k,q]: 1 where k > q
    make_lower_triangular(nc, maskT[:], val=1.0, diag=False)

    # warmup stream: keep the PE HAM clock gate busy during the DMA prologue
    with tc.tile_pool(name=pfx + "warmps", bufs=1, space="PSUM") as warmps:
        ps_w = warmps.tile([P, SCH], f32, tag="ps_w")
        for _w in range(NWARM):
            nc.tensor.matmul(ps_w[:], warm_w[:], zeros_r[:],
                             start=True, stop=True)
        # pin the warm chain against dead-code elimination: writes 0.0 over
        # maskT[0,0] which is 0.0 anyway (diagonal is unmasked); tri01 below
        # reads maskT, keeping this copy (and thus the warm matmuls) alive
        nc.vector.tensor_copy(maskT[0:1, 0:1], ps_w[0:1, 0:1])

    maskBig = consts.tile([P, P], f32, tag="maskBig")  # [k,q]: -1e30 if k > q
    nc.vector.tensor_scalar_mul(maskBig[:], maskT[:], -1e30)

    # ---- persistent activations ----
    # qp{pr}: [h_{2pr} r(32); h_{2pr} i(32); h_{2pr+1} r(32); h_{2pr+1} i(32)]
    qp0 = persist.tile([P, S], bf16, tag="qp0")
    qp1 = persist.tile([P, S], bf16, tag="qp1")
    k2 = persist.tile([P, S], bf16, tag="k2")        # [k_r; k_i] x2
    v_ones = persist.tile([P, NSBLK, HD + 1], bf16, tag="v_ones")  # [k, kb, 65]
    vT_sb = persist.tile([64, S], f32r, tag="vT_sb")
    attn_T0 = persist.tile([P, S], bf16, tag="attn_T0")  # heads 0,1
    attn_T1 = persist.tile([P, S], bf16, tag="attn_T1")  # heads 2,3

    nc.vector.tensor_copy(v_ones[:, :, HD:HD + 1],
                          ones32[:, None, :].to_broadcast((P, NSBLK, 1)))

    # ================= Phase A: QKV projections + rope =================
    if "A" not in phases:
        return
    with (
        tc.tile_pool(name=pfx + "xtsb", bufs=2) as xt_pool,
        tc.tile_pool(name=pfx + "ropetmp", bufs=2) as rtmp_pool,
        tc.tile_pool(name=pfx + "qstage", bufs=2) as qst_pool,
        tc.tile_pool(name=pfx + "psA", bufs=2, space="PSUM") as psA,
        tc.tile_pool(name=pfx + "psAq", bufs=2, space="PSUM") as psAq,
    ):
        for sch in range(NSCH):
            s0 = sch * SCH
            xt_c = xt_pool.tile([P, NDBLK, SCH], bf16, tag="xt_c")
            if sch == 0:
                # quarters: finer granularity so the first matmuls start early
                nc.sync.dma_start(xt_c[:, 0:4, :], xt_d[:, 0, 0:4])
                nc.scalar.dma_start(xt_c[:, 4:8, :], xt_d[:, 0, 4:8])
                nc.sync.dma_start(xt_c[:, 8:12, :], xt_d[:, 0, 8:12])
                nc.scalar.dma_start(xt_c[:, 12:16, :], xt_d[:, 0, 12:16])
                nc.scalar.dma_start(wo_sb[:], wo_d[:])
            else:
                nc.sync.dma_start(xt_c[:, 0:8, :], xt_d[:, sch, 0:8])
                nc.scalar.dma_start(xt_c[:, 8:16, :], xt_d[:, sch, 8:16])

            ps_q = psAq.tile([P, 2, SCH], f32, tag="ps_q")
            ps_kv = psAq.tile([P, SCH], f32, tag="ps_kv")
            for db in range(NDBLK):
                st = db == 0
                sp = db == NDBLK - 1
                nc.tensor.matmul(ps_q[:, 0, :], wq_r[:, db, :], xt_c[:, db, :],
                                 start=st, stop=sp)
                nc.tensor.matmul(ps_q[:, 1, :], wq_i[:, db, :], xt_c[:, db, :],
                                 start=st, stop=sp)
                nc.tensor.matmul(ps_kv[:], wkvi[:, db, :], xt_c[:, db, :],
                                 start=st, stop=sp)

            ssl = slice(s0, s0 + SCH)
            ps_qr = ps_q[:, 0, :]
            ps_qi = ps_q[:, 1, :]
            # ---- rope q (split layout: all 4 heads' r parts / i parts) ----
            qst_r = qst_pool.tile([P, SCH], bf16, tag="qst_r")
            qst_i = qst_pool.tile([P, SCH], bf16, tag="qst_i")
            ta = rtmp_pool.tile([P, SCH], f32, tag="ta")
            tb = rtmp_pool.tile([P, SCH], f32, tag="tb")
            nc.vector.tensor_mul(ta[:], ps_qr, cosT4[:, ssl])
            nc.vector.tensor_mul(tb[:], ps_qi, sinT4[:, ssl])
            nc.vector.tensor_sub(qst_r[:], ta[:], tb[:])
            tc2 = rtmp_pool.tile([P, SCH], f32, tag="tc2")
            td = rtmp_pool.tile([P, SCH], f32, tag="td")
            nc.vector.tensor_mul(tc2[:], ps_qr, sinT4[:, ssl])
            nc.vector.tensor_mul(td[:], ps_qi, cosT4[:, ssl])
            nc.vector.tensor_add(qst_i[:], tc2[:], td[:])
            # interleave into qp tiles: [h r; h i] per head (DMA)
            for h in range(HPC):
                qp = qp0 if h < 2 else qp1
                b = 64 * (h % 2)
                nc.gpsimd.dma_start(qp[b:b + 32, ssl],
                                    qst_r[32 * h:32 * h + 32, :])
                nc.gpsimd.dma_start(qp[b + 32:b + 64, ssl],
                                    qst_i[32 * h:32 * h + 32, :])

            # ---- rope k (kv psum rows 0:64 = [k_e, k_o]) ----
            kr = rtmp_pool.tile([32, SCH], f32, tag="kr")
            ki = rtmp_pool.tile([32, SCH], f32, tag="ki")
            nc.vector.tensor_copy(kr[:], ps_kv[0:32, :])
            nc.vector.tensor_copy(ki[:], ps_kv[32:64, :])
            tka = rtmp_pool.tile([32, SCH], f32, tag="tka")
            tkb = rtmp_pool.tile([32, SCH], f32, tag="tkb")
            nc.vector.tensor_mul(tka[:], kr[:], cosT4[0:32, ssl])
            nc.vector.tensor_mul(tkb[:], ki[:], sinT4[0:32, ssl])
            nc.vector.tensor_sub(k2[0:32, ssl], tka[:], tkb[:])
            nc.vector.tensor_mul(tka[:], kr[:], sinT4[0:32, ssl])
            nc.vector.tensor_mul(tkb[:], ki[:], cosT4[0:32, ssl])
            nc.vector.tensor_add(k2[32:64, ssl], tka[:], tkb[:])
            # replicate [k_r; k_i] to rows 64:128 (DMA)
            nc.gpsimd.dma_start(k2[64:128, ssl], k2[0:64, ssl])
            # stash vT
            nc.scalar.copy(vT_sb[:, ssl], ps_kv[64:128, :])
            # ---- v natural ([k,65] blocks with ones col) via PE transpose ----
            for kb in range(4 * sch, 4 * sch + 4):
                ps_v = psA.tile([P, 64], f32, tag="ps_v")
                nc.tensor.transpose(ps_v[:].bitcast(f32r),
                                    vT_sb[:, kb * P:(kb + 1) * P],
                                    ident[0:64, 0:64])
                nc.vector.tensor_copy(v_ones[:, kb, 0:HD], ps_v[:])

    # ====== Phase B: attention, software-pipelined, out_proj interleaved ======
    if "B" not in phases:
        return
    engs = [nc.sync, nc.scalar, nc.gpsimd]
    with (
        tc.tile_pool(name=pfx + "expt", bufs=3) as exp_pool,
        tc.tile_pool(name=pfx + "norm", bufs=4) as norm_pool,
        tc.tile_pool(name=pfx + "osb", bufs=4) as out_pool,
        tc.tile_pool(name=pfx + "psB", bufs=2, space="PSUM") as psB,
        tc.tile_pool(name=pfx + "psBo", bufs=3, space="PSUM") as psBo,
        tc.tile_pool(name=pfx + "psC", bufs=1, space="PSUM") as psC,
    ):
        def emit_c_block(qsb_c, j, pool):
            sb = 4 * qsb_c + j // 4
            dmc = j % 4
            ssl = slice(sb * P, (sb + 1) * P)
            dsl = slice(dmc * 512, (dmc + 1) * 512)
            ps_o = pool.tile([P, 512], f32, tag="ps_o", name=f"ps_o{sb}_{dmc}")
            nc.tensor.matmul(ps_o[:], attn_T0[:, ssl], wo_sb[:, 0, dsl],
                             start=True, stop=False)
            nc.tensor.matmul(ps_o[:], attn_T1[:, ssl], wo_sb[:, 1, dsl],
                             start=False, stop=True)
            osb = out_pool.tile([P, 512], bf16, tag="osb", name=f"osb{sb}_{dmc}")
            nc.vector.tensor_copy(osb[:], ps_o[:])
            engs[(4 * sb + dmc) % 3].dma_start(out_d[sb, dmc], osb[:])

        for qsb in range(NQSB):
            q0 = qsb * QSB
            nkb = (q0 + QSB) // P
            qsl = slice(q0, q0 + QSB)
            n_c = 16 if qsb > 0 else 0   # out_proj blocks of superblock qsb-1
            ci = 0
            for pr in range(2):                     # head pairs (0,1), (2,3)
                qp = qp0 if pr == 0 else qp1
                outps = [psBo.tile([HD + 1, QSB], f32, tag="outp",
                                   name=f"outp{qsb}_{pr}_{_m}") for _m in range(2)]
                prev = None                         # (kb, off, expT)
                for kb in range(nkb):
                    k0 = kb * P
                    ksl = slice(k0, k0 + P)
                    off = max(0, k0 - q0)
                    diag = k0 - q0 >= 0
                    # causal: columns [0:off] are strictly above the diagonal
                    scT = psB.tile([P, 2, QSB], f32, tag="scT")
                    for m in range(2):
                        rp = slice(64 * m, 64 * m + 64)
                        nc.tensor.matmul(scT[:, m, off:], k2[rp, ksl],
                                         qp[rp, q0 + off:q0 + QSB],
                                         start=True, stop=True,
                                         tile_position=(64 * m, 0))
                    expT = exp_pool.tile([P, 2, QSB], bf16, tag="expT")
                    if diag:
                        nc.vector.tensor_add(
                            scT[:, :, off:off + P], scT[:, :, off:off + P],
                            maskBig[:, None, :].to_broadcast((P, 2, P)))
                    nc.scalar.activation(expT[:, :, off:], scT[:, :, off:],
                                         Exp, scale=SCALE)
                    # PE gap filler while the exp for this kb runs on ACT
                    if ci < n_c:
                        emit_c_block(qsb - 1, ci, psC)
                        ci += 1
                    if prev is not None:
                        pkb, poff, pexp = prev
                        for m in range(2):
                            nc.tensor.matmul(outps[m][:, poff:],
                                             v_ones[:, pkb, :],
                                             pexp[:, m, poff:],
                                             start=(pkb == 0),
                                             stop=False,
                                             skip_group_check=True)
                    prev = (kb, off, expT)
                pkb, poff, pexp = prev
                for m in range(2):
                    nc.tensor.matmul(outps[m][:, poff:], v_ones[:, pkb, :],
                                     pexp[:, m, poff:],
                                     start=(pkb == 0), stop=True,
                                     skip_group_check=True)
                # normalize + place into attn_T
                for m in range(2):
                    lrow = norm_pool.tile([1, QSB], f32, tag="lrow")
                    nc.vector.tensor_copy(lrow[:], outps[m][HD:HD + 1, :])
                    recip = norm_pool.tile([1, QSB], f32, tag="recip")
                    nc.vector.reciprocal_approx_fast(recip[:], lrow[:])
                    bcast = norm_pool.tile([64, QSB], f32, tag="bcast")
                    nc.gpsimd.partition_broadcast(bcast[:], recip[:])
                    dst = attn_T0 if pr == 0 else attn_T1
                    rsl = slice(64 * m, 64 * m + 64)
                    nc.vector.tensor_mul(dst[rsl, qsl], outps[m][0:HD, :],
                                         bcast[:])
            while ci < n_c:
                emit_c_block(qsb - 1, ci, psC)
                ci += 1

    # ---- out_proj tail: last superblock's blocks, own deeper psum pool ----
    with (
        tc.tile_pool(name=pfx + "osb2", bufs=4) as out_pool,
        tc.tile_pool(name=pfx + "psC2", bufs=4, space="PSUM") as psC2,
    ):
        def emit_c_tail(j):
            sb = 4 * (NQSB - 1) + j // 4
            dmc = j % 4
            ssl = slice(sb * P, (sb + 1) * P)
            dsl = slice(dmc * 512, (dmc + 1) * 512)
            ps_o = psC2.tile([P, 512], f32, tag="ps_o", name=f"ps_oT{sb}_{dmc}")
            nc.tensor.matmul(ps_o[:], attn_T0[:, ssl], wo_sb[:, 0, dsl],
                             start=True, stop=False)
            nc.tensor.matmul(ps_o[:], attn_T1[:, ssl], wo_sb[:, 1, dsl],
                             start=False, stop=True)
            osb = out_pool.tile([P, 512], bf16, tag="osb", name=f"osbT{sb}_{dmc}")
            if dmc % 2 == 0:
                nc.vector.tensor_copy(osb[:], ps_o[:])
            else:
                nc.scalar.copy(osb[:], ps_o[:])
            engs[(4 * sb + dmc) % 3].dma_start(out_d[sb, dmc], osb[:])

        for j in range(16):
            emit_c_tail(j)
    tc.strict_bb_all_engine_barrier()


_NC_CACHE = {}


def _get_nc(reps=1, phases="ABEPNC"):
    key = (reps, phases)
    if key not in _NC_CACHE:
        _NC_CACHE[key] = _build_kernel(reps, phases)
    return _NC_CACHE[key]


def _make_in_maps(x, wq, wk, wv, wo, freqs_cos, freqs_sin):
    import ml_dtypes
    bf = ml_dtypes.bfloat16
    x2 = np.asarray(x, dtype=np.float32).reshape(S, D)
    # [D, S] -> [p, sch, o, s] fully contiguous per partition
    xT = np.ascontiguousarray(
        x2.T.reshape(NDBLK, P, NSCH, SCH).transpose(1, 2, 0, 3).astype(bf))
    cos = np.asarray(freqs_cos, dtype=np.float32)
    sin = np.asarray(freqs_sin, dtype=np.float32)
    cosT4 = np.ascontiguousarray(np.tile(cos.T, (HPC, 1)).astype(bf))  # [128,S]
    sinT4 = np.ascontiguousarray(np.tile(sin.T, (HPC, 1)).astype(bf))
    wq = np.asarray(wq, dtype=np.float32)
    wk = np.asarray(wk, dtype=np.float32)
    wv = np.asarray(wv, dtype=np.float32)
    wo = np.asarray(wo, dtype=np.float32)

    def _blk(w):  # [D, 128] -> [p, o, m]
        return np.ascontiguousarray(
            w.reshape(NDBLK, P, P).transpose(1, 0, 2).astype(bf))

    in_maps = []
    for c in range(NCORES):
        wq_c = wq.reshape(D, NH, HD)[:, HPC * c:HPC * (c + 1), :]
        wq_r = _blk(wq_c[:, :, 0::2].reshape(D, HPC * D2))
        wq_i = _blk(wq_c[:, :, 1::2].reshape(D, HPC * D2))
        wk_c = wk.reshape(D, NKV, HD)[:, c, :]
        wv_c = wv.reshape(D, NKV, HD)[:, c, :]
        wkvi = _blk(np.concatenate([wk_c[:, 0::2], wk_c[:, 1::2], wv_c], axis=1))
        wo_c = np.ascontiguousarray(
            wo.reshape(NH, HD, D)[HPC * c:HPC * (c + 1)]
            .reshape(2, P, D).astype(bf).transpose(1, 0, 2))
        in_maps.append({
            "xT": xT, "wq_r": wq_r, "wq_i": wq_i, "wkvi": wkvi,
            "wo_c": wo_c, "cosT4": cosT4, "sinT4": sinT4,
        })
    return in_maps


_last_in_maps = None


def kernel(x, wq, wk, wv, wo, freqs_cos, freqs_sin, mask):
    global _last_in_maps
    in_maps = _make_in_maps(x, wq, wk, wv, wo, freqs_cos, freqs_sin)
    _last_in_maps = in_maps
    nc = _get_nc()
    res = bass_utils.run_bass_kernel_spmd(nc, in_maps, core_ids=list(range(NCORES)))
    out = np.zeros((NSBLK, 4, P, 512), dtype=np.float64)
    for r in res.results:
        out += r["out"].astype(np.float64)
    out = out.transpose(0, 2, 1, 3).reshape(S, D)
    return out.astype(np.float32).reshape(1, S, D)
